# revision 1
# baseline (speedup 1.0000x reference)
"""Trainium2 Bass kernel for MinimalKAN forward (nn_MinimalKAN_Normalized).

Math:
  a = sigmoid(alpha)
  out = (1-a) * (x @ W.T + b) + (a/sqrt(I)) * (x @ C0 + x^2 @ C1 + x^3 @ C2)

Folding the alpha blend into the weights on the host gives exactly
  out = x @ A + x^2 @ B + x^3 @ C + b_eff
with A = (1-a) W.T + s C0, B = s C1, C = s C2, b_eff = (1-a) b, s = a/sqrt(I).

Device strategy (data-parallel over batch, 8 cores), per core shard 4096 rows:
  The contraction index i must sit on SBUF partitions for the TensorEngine,
  so the kernel consumes x^T.  Host mode (default) feeds x^T per core and the
  device runs pure matmuls; device mode PE-transposes x tiles via identity.
  Per 512-row batch group:
    - DMA x^T group [128, 4, 512] (f32r)
    - ACT: x2 = Square(x), DVE: x3 = x2*x  (group-batched, f32r)
    - per 128-row tile: 12 accumulating f32r matmuls into one PSUM bank
        (lhsT = basis^T k-slice [128,128], rhs = weight slice [128,512])
    - DVE: bias add fused into the PSUM->SBUF copy
    - DMA out group [128, 4, 512] on the ACT HWDGE ring
Matmul dtype (KAN_MM_DTYPE): float16 default — fp16 weights are host-scaled
by WSCALE=64 to clear the subnormal range and the PSUM result is rescaled in
the fused bias-add; ~2.4e-4 rel error at 1 cyc/row.  float32r: ~1.2e-4 rel
error but ~1.22 cyc/row (moving-operand SBUF bandwidth); float32: exact, 4x
slower.  fp32/f32r mixing with 16-bit operands is rejected by the hardware.
"""

import os
import numpy as np

import concourse.bass as bass
from concourse import bacc
import concourse.mybir as mybir
import concourse.tile as tile
from concourse.bass_utils import run_bass_kernel_spmd
from concourse.masks import make_identity

N_CORES = 8
B, I, O = 32768, 512, 512
BS = B // N_CORES          # rows per core
P = 128
N_TILES = BS // P          # 32 tiles per core
KS = I // P                # 4 contraction slices per basis

_MM_DTYPE = os.environ.get("KAN_MM_DTYPE", "float16")
_HOST_T = os.environ.get("KAN_HOST_T", "1") == "1"
_GROUP = int(os.environ.get("KAN_GROUP", "4"))
_W_BF16 = os.environ.get("KAN_W_BF16", "0") == "1"


WSCALE = 64.0  # host multiplies fp16 weights by this to stay in normal range


def _build(mm_dtype_name: str, repeat: int = 1, host_t: bool = _HOST_T,
           group: int = _GROUP, w_bf16: bool = _W_BF16) -> bass.Bass:
    mm_dt = getattr(mybir.dt, mm_dtype_name)
    w_dt = mybir.dt.bfloat16 if w_bf16 else mm_dt
    if mm_dtype_name in ("float32r", "float16"):
        x_dt = mm_dt
    else:
        x_dt = mybir.dt.float32
    G = group
    GB = G * P                     # batch rows per group
    n_groups = N_TILES // G
    sq = mybir.ActivationFunctionType.Square

    nc = bacc.Bacc("TRN2", target_bir_lowering=False, debug=False,
                   num_devices=N_CORES)

    if host_t:
        x_d = nc.dram_tensor("xt", [I, BS], x_dt, kind="ExternalInput")
        x_r = x_d.rearrange("(ks p) b -> p ks b", p=P)
    else:
        x_d = nc.dram_tensor("x", [BS, I], x_dt, kind="ExternalInput")
        x_g = x_d.rearrange("(g a p) k -> g p a k", a=G, p=P)
    w_d = nc.dram_tensor("wcat", [3 * I, O], w_dt, kind="ExternalInput")
    b_d = nc.dram_tensor("bias", [P, O], mybir.dt.float32,
                         kind="ExternalInput")
    o_d = nc.dram_tensor("out", [BS, O], mybir.dt.float32,
                         kind="ExternalOutput")
    o_g = o_d.rearrange("(g a p) k -> g p a k", a=G, p=P)

    w_r = w_d.rearrange("(ks p) o -> p ks o", p=P)

    with tile.TileContext(nc) as tc:
        with (
            tc.tile_pool(name="const", bufs=1) as const,
            tc.tile_pool(name="xin", bufs=3) as xin,
            tc.tile_pool(name="xt", bufs=4) as xt,
            tc.tile_pool(name="outp", bufs=4) as outp,
            tc.tile_pool(name="psum_t", bufs=3, space="PSUM") as psum_t,
            tc.tile_pool(name="psum_o", bufs=6, space="PSUM") as psum_o,
        ):
            if not host_t:
                if x_dt == mybir.dt.float32:
                    ident = const.tile([P, P], x_dt)
                    make_identity(nc, ident[:])
                else:
                    ident_f32 = const.tile([P, P], mybir.dt.float32)
                    make_identity(nc, ident_f32[:])
                    ident = const.tile([P, P], x_dt)
                    nc.vector.tensor_copy(out=ident[:], in_=ident_f32[:])

            wsb = const.tile([P, 3 * KS, O], w_dt)
            for ws in range(3 * KS):
                nc.sync.dma_start(wsb[:, ws, :], w_r[:, ws, :])
            bsb = const.tile([P, O], mybir.dt.float32)
            nc.sync.dma_start(bsb[:], b_d[:, :])

            for g in [i for _ in range(repeat) for i in range(n_groups)]:
                if host_t:
                    xT = xt.tile([P, KS, GB], mm_dt, tag="xT")
                    nc.sync.dma_start(xT[:], x_r[:, :, g * GB:(g + 1) * GB])
                else:
                    x_sb = xin.tile([P, G, I], x_dt, tag="x_sb")
                    nc.sync.dma_start(x_sb[:], x_g[g])
                    xT = xt.tile([P, KS, GB], mm_dt, tag="xT")
                    for j in range(G):
                        pt = psum_t.tile([P, KS, P], x_dt, tag="pt")
                        for k in range(KS):
                            nc.tensor.transpose(
                                pt[:, k, :],
                                x_sb[:, j, k * P:(k + 1) * P], ident[:])
                        for k in range(KS):
                            nc.vector.tensor_copy(
                                out=xT[:, k, j * P:(j + 1) * P],
                                in_=pt[:, k, :])

                x2T = xt.tile([P, KS, GB], mm_dt, tag="x2T")
                x3T = xt.tile([P, KS, GB], mm_dt, tag="x3T")
                o_sb = outp.tile([P, G, O], mybir.dt.float32, tag="o_sb")
                PING = os.environ.get("KAN_PINGPONG", "0") == "1"
                for j0 in range(0, G, 2 if PING else 1):
                    jset = [j0, j0 + 1] if PING else [j0]
                    pos = []
                    for j in jset:
                        js = slice(j * P, (j + 1) * P)
                        nc.scalar.activation(x2T[:, :, js], xT[:, :, js], sq)
                        nc.vector.tensor_mul(x3T[:, :, js], x2T[:, :, js],
                                             xT[:, :, js])
                        po_t = psum_o.tile([P, O], mybir.dt.float32,
                                           tag="po", name="po")
                        pos.append(po_t)
                    idx = 0
                    for bi, XT in enumerate((xT, x2T, x3T)):
                        for k in range(KS):
                            for j, po in zip(jset, pos):
                                nc.tensor.matmul(
                                    po[:],
                                    XT[:, k, j * P:(j + 1) * P],
                                    wsb[:, bi * KS + k, :],
                                    start=(idx == 0),
                                    stop=(idx == 3 * KS - 1),
                                    skip_group_check=True,
                                )
                            idx += 1
                    for j, po in zip(jset, pos):
                        if mm_dtype_name == "float16":
                            nc.vector.scalar_tensor_tensor(
                                o_sb[:, j, :], po[:], 1.0 / WSCALE, bsb[:],
                                mybir.AluOpType.mult, mybir.AluOpType.add)
                        else:
                            nc.vector.tensor_add(o_sb[:, j, :], po[:],
                                                 bsb[:])
                if os.environ.get("KAN_SKIP_OUT", "0") != "1":
                    nc.scalar.dma_start(o_g[g], o_sb[:])

    nc.compile()
    return nc


_NC_CACHE: dict[str, bass.Bass] = {}


def _get_nc() -> bass.Bass:
    nc = _NC_CACHE.get(_MM_DTYPE)
    if nc is None:
        nc = _build(_MM_DTYPE)
        _NC_CACHE[_MM_DTYPE] = nc
    return nc


def _fold_weights(coeffs, W, b, alpha):
    a = 1.0 / (1.0 + np.exp(-np.float64(alpha)))
    s = a / np.sqrt(np.float64(I))
    A = (1.0 - a) * W.astype(np.float64).T + s * coeffs[:, :, 0].astype(np.float64)
    Bm = s * coeffs[:, :, 1].astype(np.float64)
    Cm = s * coeffs[:, :, 2].astype(np.float64)
    wcat = np.ascontiguousarray(
        np.concatenate([A, Bm, Cm], axis=0).astype(np.float32))
    b_eff = ((1.0 - a) * b.astype(np.float64)).astype(np.float32)
    bias_rep = np.ascontiguousarray(
        np.broadcast_to(b_eff[None, :], (P, O)).astype(np.float32))
    return wcat, bias_rep


def _make_in_maps(x, coeffs, W, b, alpha):
    wcat, bias_rep = _fold_weights(coeffs, W, b, alpha)
    if _MM_DTYPE == "bfloat16" or _W_BF16:
        import ml_dtypes
        wcat = wcat.astype(ml_dtypes.bfloat16)
    elif _MM_DTYPE == "float16":
        wcat = (wcat.astype(np.float64) * WSCALE).astype(np.float16)
    x = np.asarray(x, dtype=np.float32)
    in_maps = []
    for c in range(N_CORES):
        shard = x[c * BS:(c + 1) * BS]
        m = {"wcat": wcat, "bias": bias_rep}
        x_np = np.float16 if _MM_DTYPE == "float16" else np.float32
        if _HOST_T:
            m["xt"] = np.ascontiguousarray(shard.T.astype(x_np))
        else:
            m["x"] = np.ascontiguousarray(shard.astype(x_np))
        in_maps.append(m)
    return in_maps


def _run(x, coeffs, W, b, alpha, trace=False):
    nc = _get_nc()
    in_maps = _make_in_maps(x, coeffs, W, b, alpha)
    res = run_bass_kernel_spmd(nc, in_maps, core_ids=list(range(N_CORES)),
                               trace=trace)
    out = np.concatenate([r["out"] for r in res.results], axis=0)
    return out, res


def kernel(x, coeffs, W, b, alpha):
    out, _ = _run(x, coeffs, W, b, alpha, trace=False)
    return out



# revision 8
# speedup vs baseline: 1.4630x; 1.4630x over previous
"""Trainium2 Bass kernel for MinimalKAN forward (nn_MinimalKAN_Normalized).

Math:
  a = sigmoid(alpha)
  out = (1-a) * (x @ W.T + b) + (a/sqrt(I)) * (x @ C0 + x^2 @ C1 + x^3 @ C2)

Folding the alpha blend into the weights on the host gives exactly
  out = x @ A + x^2 @ B + x^3 @ C + b_eff
with A = (1-a) W.T + s C0, B = s C1, C = s C2, b_eff = (1-a) b, s = a/sqrt(I).

Device strategy (data-parallel over batch, 8 cores), per core 4096 rows.
The contraction index i sits on SBUF partitions; the host feeds x^T in fp16.
Mixed precision split by term magnitude:
  - linear term x @ A: fp16 matmuls (A host-scaled by S16=64 to clear the
    fp16 subnormal range), 4 accumulating matmuls per 128-row tile.
  - kan terms x^2 @ B + x^3 @ C: fp8(e4m3) matmuls in DoubleRow perf mode
    (two k-tiles per instruction, 0.5 cyc/col, 2x fp16 MAC rate; out tile
    is 64 partitions, so each 128-row tile runs as two 64-row halves into
    one PSUM bank at tile_position col 0/64).  B,C are tiny (~2e-4) so fp8
    error lands well under the tolerance; host-scaled by 4096/8192 to
    clear fp8 subnormals, and the x^3 stream is halved on device to stay
    below TRN-e4m3 saturation (+-240; |x|<6.2 -> |x^3/2|<120).
  - x^2 on ACT (Square, fp8 out), x^3/2 on GpSimd ((x^2*0.5)*x), PSUM
    merges + bias on DVE, output stored bf16 (halves output DMA traffic).
Per 128-row tile PE cost: 4*512 (fp16) + 8*256 (fp8 DR) = 4096 cycles vs
12*512 = 6144 all-fp16.
"""

import os
import numpy as np

import concourse.bass as bass
from concourse import bacc
import concourse.mybir as mybir
import concourse.tile as tile
from concourse.bass_utils import run_bass_kernel_spmd

N_CORES = 8
B, I, O = 32768, 512, 512
BS = B // N_CORES          # rows per core
P = 128
KS = I // P                # 4 contraction k-tiles per basis
G = int(os.environ.get("KAN_GROUP", "4"))     # 128-row tiles per group
GB = G * P                 # batch rows per group
N_GROUPS = BS // GB

S16 = 64.0                 # fp16 linear-weight host scale
S8A = 4096.0               # fp8 kan-weight host scale (x^2 and x^3 blocks)


def _build(repeat: int = 1) -> bass.Bass:
    f16 = mybir.dt.float16
    f8 = mybir.dt.float8e4
    f32 = mybir.dt.float32
    bf16 = mybir.dt.bfloat16
    sq = mybir.ActivationFunctionType.Square
    DR = mybir.MatmulPerfMode.DoubleRow
    mult = mybir.AluOpType.mult
    add = mybir.AluOpType.add

    nc = bacc.Bacc("TRN2", target_bir_lowering=False, debug=False,
                   num_devices=N_CORES)

    x_d = nc.dram_tensor("xt", [I, BS], f16, kind="ExternalInput")
    x_r = x_d.rearrange("(ks p) b -> p ks b", p=P)
    wl_d = nc.dram_tensor("wlin", [I, O], f16, kind="ExternalInput")
    wl_r = wl_d.rearrange("(ks p) o -> p ks o", p=P)
    wk_d = nc.dram_tensor("wkan", [2 * I, O], f8, kind="ExternalInput")
    wk_r = wk_d.rearrange("(ks p) o -> p ks o", p=P)
    b_d = nc.dram_tensor("bias", [P, O], f32, kind="ExternalInput")
    o_d = nc.dram_tensor("out", [BS, O], bf16, kind="ExternalOutput")
    o_g = o_d.rearrange("(g a p) k -> g p a k", a=G, p=P)

    with tile.TileContext(nc) as tc:
        with (
            tc.tile_pool(name="const", bufs=1) as const,
            tc.tile_pool(name="xin", bufs=3) as xin,
            tc.tile_pool(name="basis", bufs=3) as basis,
            tc.tile_pool(name="outp", bufs=3) as outp,
            tc.tile_pool(name="tmp", bufs=4) as tmpp,
            tc.tile_pool(name="ps_l", bufs=3, space="PSUM") as ps_l,
            tc.tile_pool(name="ps_k", bufs=2, space="PSUM") as ps_k,
        ):
            wl_sb = const.tile([P, KS, O], f16)
            for k in range(KS):
                nc.sync.dma_start(wl_sb[:, k, :], wl_r[:, k, :])
            wk_sb = const.tile([P, 2 * KS, O], f8)
            for k in range(2 * KS):
                nc.sync.dma_start(wk_sb[:, k, :], wk_r[:, k, :])
            bsb = const.tile([P, O], f32)
            nc.sync.dma_start(bsb[:], b_d[:, :])

            for g in [i for _ in range(repeat) for i in range(N_GROUPS)]:
                xT = xin.tile([P, KS, GB], f16, tag="xT")
                nc.sync.dma_start(xT[:], x_r[:, :, g * GB:(g + 1) * GB])
                b8 = basis.tile([P, 2 * KS, GB], f8, tag="b8")
                o_sb = outp.tile([P, G, O], bf16, tag="o_sb")
                for j in range(G):
                    js = slice(j * P, (j + 1) * P)
                    nc.scalar.activation(b8[:, 0:KS, js], xT[:, :, js], sq)
                    nc.gpsimd.tensor_mul(
                        b8[:, KS:2 * KS, js], b8[:, 0:KS, js], xT[:, :, js])
                    po_l = ps_l.tile([P, O], f32, tag="po_l")
                    for k in range(KS):
                        nc.tensor.matmul(
                            po_l[:], xT[:, k, js], wl_sb[:, k, :],
                            start=(k == 0), stop=(k == KS - 1),
                            skip_group_check=True)
                    # DoubleRow matmuls can only write PSUM partitions 0-63
                    # (col_grp 0xf -> dst partition 0), so each 128-row tile
                    # runs as two 64-row halves into partition-0 psum tiles;
                    # the h=1 merge reads psum partitions 0-63 while writing
                    # SBUF partitions 64-127.
                    po_k = [ps_k.tile([64, O], f32, tag=f"po_k{h}",
                                      name=f"po_k{h}")
                            for h in range(2)]
                    for h in range(2):
                        hs = slice(j * P + h * 64, j * P + (h + 1) * 64)
                        for t in range(KS):
                            nc.tensor.matmul(
                                po_k[h][:],
                                b8[:, 2 * t:2 * t + 2, hs],
                                wk_sb[:, 2 * t:2 * t + 2, :],
                                start=(t == 0), stop=(t == KS - 1),
                                perf_mode=DR, skip_group_check=True)
                    tmp = tmpp.tile([P, O], f32, tag="tmp")
                    nc.vector.scalar_tensor_tensor(
                        tmp[:], po_l[:], 1.0 / S16, bsb[:], mult, add)
                    for h in range(2):
                        nc.vector.scalar_tensor_tensor(
                            o_sb[h * 64:(h + 1) * 64, j, :], po_k[h][:],
                            1.0 / S8A, tmp[h * 64:(h + 1) * 64, :], mult, add)
                nc.scalar.dma_start(o_g[g], o_sb[:])

    nc.compile()
    return nc


_NC_CACHE: dict[int, bass.Bass] = {}


def _get_nc(repeat: int = 1) -> bass.Bass:
    nc = _NC_CACHE.get(repeat)
    if nc is None:
        nc = _build(repeat)
        _NC_CACHE[repeat] = nc
    return nc


def _fold_weights(coeffs, W, b, alpha):
    a = 1.0 / (1.0 + np.exp(-np.float64(alpha)))
    s = a / np.sqrt(np.float64(I))
    A = (1.0 - a) * W.astype(np.float64).T + s * coeffs[:, :, 0].astype(np.float64)
    Bm = s * coeffs[:, :, 1].astype(np.float64)
    Cm = s * coeffs[:, :, 2].astype(np.float64)
    wlin = (A * S16).astype(np.float16)
    f8np = mybir.dt.np(mybir.dt.float8e4)
    wkan = np.concatenate([Bm * S8A, Cm * S8A], axis=0)
    wkan = np.ascontiguousarray(np.clip(wkan, -240.0, 240.0)).astype(f8np)
    b_eff = ((1.0 - a) * b.astype(np.float64)).astype(np.float32)
    bias_rep = np.ascontiguousarray(
        np.broadcast_to(b_eff[None, :], (P, O)).astype(np.float32))
    return np.ascontiguousarray(wlin), wkan, bias_rep


def _make_in_maps(x, coeffs, W, b, alpha):
    wlin, wkan, bias_rep = _fold_weights(coeffs, W, b, alpha)
    x = np.asarray(x, dtype=np.float32)
    in_maps = []
    for c in range(N_CORES):
        shard = x[c * BS:(c + 1) * BS]
        in_maps.append({
            "wlin": wlin, "wkan": wkan, "bias": bias_rep,
            "xt": np.ascontiguousarray(shard.T.astype(np.float16)),
        })
    return in_maps


def _run(x, coeffs, W, b, alpha, trace=False):
    nc = _get_nc()
    in_maps = _make_in_maps(x, coeffs, W, b, alpha)
    res = run_bass_kernel_spmd(nc, in_maps, core_ids=list(range(N_CORES)),
                               trace=trace)
    out = np.concatenate(
        [np.asarray(r["out"]).astype(np.float32) for r in res.results], axis=0)
    return out, res


def kernel(x, coeffs, W, b, alpha):
    out, _ = _run(x, coeffs, W, b, alpha, trace=False)
    return out


# revision 10
# speedup vs baseline: 2.0306x; 1.3879x over previous
"""Trainium2 Bass kernel for MinimalKAN forward (nn_MinimalKAN_Normalized).

Math:
  a = sigmoid(alpha)
  out = (1-a) * (x @ W.T + b) + (a/sqrt(I)) * (x @ C0 + x^2 @ C1 + x^3 @ C2)

Folding the alpha blend into the weights on the host gives exactly
  out = x @ A + x^2 @ B + x^3 @ C + b_eff
with A = (1-a) W.T + s C0, B = s C1, C = s C2, b_eff = (1-a) b, s = a/sqrt(I).

Device strategy (data-parallel over batch, 8 cores), per core 4096 rows.
The contraction index i sits on SBUF partitions; the host feeds x^T in fp16.
Mixed precision split by term magnitude:
  - linear term x @ A: fp16 matmuls (A host-scaled by S16=64 to clear the
    fp16 subnormal range), 4 accumulating matmuls per 128-row tile.
  - kan terms x^2 @ B + x^3 @ C: fp8(e4m3) matmuls in DoubleRow perf mode
    (two k-tiles per instruction, 0.5 cyc/col, 2x fp16 MAC rate; out tile
    is 64 partitions, so each 128-row tile runs as two 64-row halves into
    one PSUM bank at tile_position col 0/64).  B,C are tiny (~2e-4) so fp8
    error lands well under the tolerance; host-scaled by 4096/8192 to
    clear fp8 subnormals, and the x^3 stream is halved on device to stay
    below TRN-e4m3 saturation (+-240; |x|<6.2 -> |x^3/2|<120).
  - x^2 on ACT (Square, fp8 out), x^3/2 on GpSimd ((x^2*0.5)*x), PSUM
    merges + bias on DVE, output stored bf16 (halves output DMA traffic).
Per 128-row tile PE cost: 4*512 (fp16) + 8*256 (fp8 DR) = 4096 cycles vs
12*512 = 6144 all-fp16.
"""

import os
import numpy as np

import concourse.bass as bass
from concourse import bacc
import concourse.mybir as mybir
import concourse.tile as tile
from concourse.bass_utils import run_bass_kernel_spmd

N_CORES = 8
B, I, O = 32768, 512, 512
BS = B // N_CORES          # rows per core
P = 128
KS = I // P                # 4 contraction k-tiles per basis
G = int(os.environ.get("KAN_GROUP", "4"))     # 128-row tiles per group
GB = G * P                 # batch rows per group
N_GROUPS = BS // GB

S16 = 64.0                 # fp16 linear-weight host scale
S8A = 4096.0               # fp8 kan-weight host scale (x^2 and x^3 blocks)


def _build(repeat: int = 1) -> bass.Bass:
    f16 = mybir.dt.float16
    f8 = mybir.dt.float8e4
    f32 = mybir.dt.float32
    bf16 = mybir.dt.bfloat16
    sq = mybir.ActivationFunctionType.Square
    DR = mybir.MatmulPerfMode.DoubleRow
    mult = mybir.AluOpType.mult
    add = mybir.AluOpType.add

    nc = bacc.Bacc("TRN2", target_bir_lowering=False, debug=False,
                   num_devices=N_CORES)

    x_d = nc.dram_tensor("xt", [I, BS], f16, kind="ExternalInput")
    x_r = x_d.rearrange("(ks p) b -> p ks b", p=P)
    wl_d = nc.dram_tensor("wlin", [I, O], f16, kind="ExternalInput")
    wl_r = wl_d.rearrange("(ks p) o -> p ks o", p=P)
    wk_d = nc.dram_tensor("wkan", [2 * I, O], f8, kind="ExternalInput")
    wk_r = wk_d.rearrange("(ks p) o -> p ks o", p=P)
    b_d = nc.dram_tensor("bias", [P, O], f32, kind="ExternalInput")
    o_d = nc.dram_tensor("out", [BS, O], bf16, kind="ExternalOutput")
    o_g = o_d.rearrange("(g a p) k -> g p a k", a=G, p=P)

    with tile.TileContext(nc) as tc:
        with (
            tc.tile_pool(name="const", bufs=1) as const,
            tc.tile_pool(name="xin", bufs=3) as xin,
            tc.tile_pool(name="basis", bufs=3) as basis,
            tc.tile_pool(name="outp", bufs=3) as outp,
            tc.tile_pool(name="tmp", bufs=4) as tmpp,
            tc.tile_pool(name="ps_l", bufs=3, space="PSUM") as ps_l,
            tc.tile_pool(name="ps_k", bufs=3, space="PSUM") as ps_k,
        ):
            wl_sb = const.tile([P, KS, O], f16)
            for k in range(KS):
                nc.sync.dma_start(wl_sb[:, k, :], wl_r[:, k, :])
            wk_sb = const.tile([P, 2 * KS, O], f8)
            for k in range(2 * KS):
                nc.sync.dma_start(wk_sb[:, k, :], wk_r[:, k, :])
            bsb = const.tile([P, O], f32)
            nc.sync.dma_start(bsb[:], b_d[:, :])

            for g in [i for _ in range(repeat) for i in range(N_GROUPS)]:
                xT = xin.tile([P, KS, GB], f16, tag="xT")
                nc.sync.dma_start(xT[:], x_r[:, :, g * GB:(g + 1) * GB])
                b8 = basis.tile([P, 2 * KS, GB], f8, tag="b8")
                o_sb = outp.tile([P, G, O], bf16, tag="o_sb")
                for j in range(G):
                    js = slice(j * P, (j + 1) * P)
                    nc.scalar.activation(b8[:, 0:KS, js], xT[:, :, js], sq)
                    nc.gpsimd.tensor_mul(
                        b8[:, KS:2 * KS, js], b8[:, 0:KS, js], xT[:, :, js])
                    po_l = ps_l.tile([P, O], f32, tag="po_l")
                    for k in range(KS):
                        nc.tensor.matmul(
                            po_l[:], xT[:, k, js], wl_sb[:, k, :],
                            start=(k == 0), stop=(k == KS - 1),
                            skip_group_check=True)
                    # DoubleRow fp8: lhsT [128, 2, 128] loads 2 k-planes
                    # (each PE cell holds 2 weights), K=256 per instruction
                    # at 1 col/cycle -> 2x the fp16 MAC rate, M=128 out.
                    po_k = ps_k.tile([P, O], f32, tag="po_k")
                    for t in range(KS):
                        nc.tensor.matmul(
                            po_k[:],
                            b8[:, 2 * t:2 * t + 2, js],
                            wk_sb[:, 2 * t:2 * t + 2, :],
                            start=(t == 0), stop=(t == KS - 1),
                            perf_mode=DR, skip_group_check=True)
                    tmp = tmpp.tile([P, O], f32, tag="tmp")
                    nc.vector.scalar_tensor_tensor(
                        tmp[:], po_l[:], 1.0 / S16, bsb[:], mult, add)
                    nc.vector.scalar_tensor_tensor(
                        o_sb[:, j, :], po_k[:], 1.0 / S8A, tmp[:], mult, add)
                nc.scalar.dma_start(o_g[g], o_sb[:])

    nc.compile()
    return nc


_NC_CACHE: dict[int, bass.Bass] = {}


def _get_nc(repeat: int = 1) -> bass.Bass:
    nc = _NC_CACHE.get(repeat)
    if nc is None:
        nc = _build(repeat)
        _NC_CACHE[repeat] = nc
    return nc


def _fold_weights(coeffs, W, b, alpha):
    a = 1.0 / (1.0 + np.exp(-np.float64(alpha)))
    s = a / np.sqrt(np.float64(I))
    A = (1.0 - a) * W.astype(np.float64).T + s * coeffs[:, :, 0].astype(np.float64)
    Bm = s * coeffs[:, :, 1].astype(np.float64)
    Cm = s * coeffs[:, :, 2].astype(np.float64)
    wlin = (A * S16).astype(np.float16)
    f8np = mybir.dt.np(mybir.dt.float8e4)
    wkan = np.concatenate([Bm * S8A, Cm * S8A], axis=0)
    wkan = np.ascontiguousarray(np.clip(wkan, -240.0, 240.0)).astype(f8np)
    b_eff = ((1.0 - a) * b.astype(np.float64)).astype(np.float32)
    bias_rep = np.ascontiguousarray(
        np.broadcast_to(b_eff[None, :], (P, O)).astype(np.float32))
    return np.ascontiguousarray(wlin), wkan, bias_rep


def _make_in_maps(x, coeffs, W, b, alpha):
    wlin, wkan, bias_rep = _fold_weights(coeffs, W, b, alpha)
    x = np.asarray(x, dtype=np.float32)
    in_maps = []
    for c in range(N_CORES):
        shard = x[c * BS:(c + 1) * BS]
        in_maps.append({
            "wlin": wlin, "wkan": wkan, "bias": bias_rep,
            "xt": np.ascontiguousarray(shard.T.astype(np.float16)),
        })
    return in_maps


def _run(x, coeffs, W, b, alpha, trace=False):
    nc = _get_nc()
    in_maps = _make_in_maps(x, coeffs, W, b, alpha)
    res = run_bass_kernel_spmd(nc, in_maps, core_ids=list(range(N_CORES)),
                               trace=trace)
    out = np.concatenate(
        [np.asarray(r["out"]).astype(np.float32) for r in res.results], axis=0)
    return out, res


def kernel(x, coeffs, W, b, alpha):
    out, _ = _run(x, coeffs, W, b, alpha, trace=False)
    return out


# revision 11
# speedup vs baseline: 2.2139x; 1.0903x over previous
"""Trainium2 Bass kernel for MinimalKAN forward (nn_MinimalKAN_Normalized).

Math:
  a = sigmoid(alpha)
  out = (1-a) * (x @ W.T + b) + (a/sqrt(I)) * (x @ C0 + x^2 @ C1 + x^3 @ C2)

Folding the alpha blend into the weights on the host gives exactly
  out = x @ A + x^2 @ B + x^3 @ C + b_eff
with A = (1-a) W.T + s C0, B = s C1, C = s C2, b_eff = (1-a) b, s = a/sqrt(I).

Device strategy (data-parallel over batch, 8 cores), per core 4096 rows.
The contraction index i sits on SBUF partitions; the host feeds x^T in fp16.
Mixed precision split by term magnitude:
  - linear term x @ A: fp16 matmuls (A host-scaled by S16=64 to clear the
    fp16 subnormal range), 4 accumulating matmuls per 128-row tile.
  - kan terms x^2 @ B + x^3 @ C: fp8(e4m3) matmuls in DoubleRow perf mode:
    lhsT [128, 2, 128] loads two k-planes (each PE cell holds 2 weights),
    K=256 per instruction at 1 col/cycle -> 2x the fp16 MAC rate.  B,C are
    tiny (~2e-4) so fp8 error lands well under tolerance; host-scaled by
    4096 to clear fp8 subnormals.  TRN e4m3 saturates at +-240: max|x|=5.4
    -> max|x^3| ~ 160, safe.
  - x^2 on ACT (Square, fp8 out), x^3 on GpSimd (x^2*x), PSUM merges +
    bias on DVE, output stored bf16 (halves output DMA traffic).
Per 128-row tile PE cost: 4*512 (fp16) + 4*512 (fp8 DR) = 4096 cycles vs
12*512 = 6144 all-fp16: ~57us PE floor at 2.3 GHz.
All HBM tensors are host-relayouted to [128 partitions, ...contiguous] so
every DMA kick is 128 descriptors of 2-4KB (descriptor-issue rate on the
two HWDGE rings limits the pipeline head/tail otherwise).  Weights ride
the ACT ring, x^T the SP ring, outputs the ACT ring.
"""

import os
import numpy as np

import concourse.bass as bass
from concourse import bacc
import concourse.mybir as mybir
import concourse.tile as tile
from concourse.bass_utils import run_bass_kernel_spmd

N_CORES = 8
B, I, O = 32768, 512, 512
BS = B // N_CORES          # rows per core
P = 128
KS = I // P                # 4 contraction k-tiles per basis
G = int(os.environ.get("KAN_GROUP", "4"))     # 128-row tiles per group
GB = G * P                 # batch rows per group
N_GROUPS = BS // GB

S16 = 64.0                 # fp16 linear-weight host scale
S8A = 4096.0               # fp8 kan-weight host scale (x^2 and x^3 blocks)


def _build(repeat: int = 1) -> bass.Bass:
    f16 = mybir.dt.float16
    f8 = mybir.dt.float8e4
    f32 = mybir.dt.float32
    bf16 = mybir.dt.bfloat16
    sq = mybir.ActivationFunctionType.Square
    DR = mybir.MatmulPerfMode.DoubleRow
    mult = mybir.AluOpType.mult
    add = mybir.AluOpType.add

    nc = bacc.Bacc("TRN2", target_bir_lowering=False, debug=False,
                   num_devices=N_CORES)

    x_d = nc.dram_tensor("xt", [P, N_GROUPS, KS, GB], f16,
                         kind="ExternalInput")
    wl_d = nc.dram_tensor("wlin", [P, KS, O], f16, kind="ExternalInput")
    wk_d = nc.dram_tensor("wkan", [P, 2 * KS, O], f8, kind="ExternalInput")
    b_d = nc.dram_tensor("bias", [P, O], f32, kind="ExternalInput")
    o_d = nc.dram_tensor("out", [P, N_GROUPS, G, O], bf16,
                         kind="ExternalOutput")

    with tile.TileContext(nc) as tc:
        with (
            tc.tile_pool(name="const", bufs=1) as const,
            tc.tile_pool(name="xin", bufs=4) as xin,
            tc.tile_pool(name="basis", bufs=3) as basis,
            tc.tile_pool(name="outp", bufs=3) as outp,
            tc.tile_pool(name="tmp", bufs=4) as tmpp,
            tc.tile_pool(name="ps_l", bufs=3, space="PSUM") as ps_l,
            tc.tile_pool(name="ps_k", bufs=3, space="PSUM") as ps_k,
        ):
            wl_sb = const.tile([P, KS, O], f16)
            nc.scalar.dma_start(wl_sb[:], wl_d[:])
            wk_sb = const.tile([P, 2 * KS, O], f8)
            nc.scalar.dma_start(wk_sb[:], wk_d[:])
            bsb = const.tile([P, O], f32)
            nc.scalar.dma_start(bsb[:], b_d[:])

            for g in [i for _ in range(repeat) for i in range(N_GROUPS)]:
                xT = xin.tile([P, KS, GB], f16, tag="xT")
                nc.sync.dma_start(xT[:], x_d[:, g])
                b8 = basis.tile([P, 2 * KS, GB], f8, tag="b8")
                o_sb = outp.tile([P, G, O], bf16, tag="o_sb")
                for j in range(G):
                    js = slice(j * P, (j + 1) * P)
                    nc.scalar.activation(b8[:, 0:KS, js], xT[:, :, js], sq)
                    nc.gpsimd.tensor_mul(
                        b8[:, KS:2 * KS, js], b8[:, 0:KS, js], xT[:, :, js])
                    po_l = ps_l.tile([P, O], f32, tag="po_l")
                    for k in range(KS):
                        nc.tensor.matmul(
                            po_l[:], xT[:, k, js], wl_sb[:, k, :],
                            start=(k == 0), stop=(k == KS - 1),
                            skip_group_check=True)
                    po_k = ps_k.tile([P, O], f32, tag="po_k")
                    for t in range(KS):
                        nc.tensor.matmul(
                            po_k[:],
                            b8[:, 2 * t:2 * t + 2, js],
                            wk_sb[:, 2 * t:2 * t + 2, :],
                            start=(t == 0), stop=(t == KS - 1),
                            perf_mode=DR, skip_group_check=True)
                    tmp = tmpp.tile([P, O], f32, tag="tmp")
                    nc.vector.scalar_tensor_tensor(
                        tmp[:], po_l[:], 1.0 / S16, bsb[:], mult, add)
                    nc.vector.scalar_tensor_tensor(
                        o_sb[:, j, :], po_k[:], 1.0 / S8A, tmp[:], mult, add)
                nc.scalar.dma_start(o_d[:, g], o_sb[:])

    nc.compile()
    return nc


_NC_CACHE: dict[int, bass.Bass] = {}


def _get_nc(repeat: int = 1) -> bass.Bass:
    nc = _NC_CACHE.get(repeat)
    if nc is None:
        nc = _build(repeat)
        _NC_CACHE[repeat] = nc
    return nc


def _fold_weights(coeffs, W, b, alpha):
    a = 1.0 / (1.0 + np.exp(-np.float64(alpha)))
    s = a / np.sqrt(np.float64(I))
    A = (1.0 - a) * W.astype(np.float64).T + s * coeffs[:, :, 0].astype(np.float64)
    Bm = s * coeffs[:, :, 1].astype(np.float64)
    Cm = s * coeffs[:, :, 2].astype(np.float64)
    # [I, O] -> [P, KS, O] with row ks*P+p on partition p, slot ks
    wlin = (A * S16).astype(np.float16)
    wlin = np.ascontiguousarray(
        wlin.reshape(KS, P, O).transpose(1, 0, 2))
    f8np = mybir.dt.np(mybir.dt.float8e4)
    wkan = np.concatenate([Bm * S8A, Cm * S8A], axis=0)
    wkan = np.clip(wkan, -240.0, 240.0).astype(f8np)
    wkan = np.ascontiguousarray(
        wkan.reshape(2 * KS, P, O).transpose(1, 0, 2))
    b_eff = ((1.0 - a) * b.astype(np.float64)).astype(np.float32)
    bias_rep = np.ascontiguousarray(
        np.broadcast_to(b_eff[None, :], (P, O)).astype(np.float32))
    return wlin, wkan, bias_rep


def _make_in_maps(x, coeffs, W, b, alpha):
    wlin, wkan, bias_rep = _fold_weights(coeffs, W, b, alpha)
    x = np.asarray(x, dtype=np.float32)
    in_maps = []
    for c in range(N_CORES):
        shard = x[c * BS:(c + 1) * BS].astype(np.float16)
        # [BS, I] -> [P, N_GROUPS, KS, GB]: xt[p, g, ks, b'] =
        # x[g*GB+b', ks*P+p]
        xt = np.ascontiguousarray(
            shard.reshape(N_GROUPS, GB, KS, P).transpose(3, 0, 2, 1))
        in_maps.append({
            "wlin": wlin, "wkan": wkan, "bias": bias_rep, "xt": xt,
        })
    return in_maps


def _unpack_out(raw):
    # [P, N_GROUPS, G, O] bf16 -> [BS, O] f32: row g*GB + j*P + p
    return np.ascontiguousarray(
        np.asarray(raw).astype(np.float32).transpose(1, 2, 0, 3)
    ).reshape(BS, O)


def _run(x, coeffs, W, b, alpha, trace=False):
    nc = _get_nc()
    in_maps = _make_in_maps(x, coeffs, W, b, alpha)
    res = run_bass_kernel_spmd(nc, in_maps, core_ids=list(range(N_CORES)),
                               trace=trace)
    out = np.concatenate([_unpack_out(r["out"]) for r in res.results], axis=0)
    return out, res


def kernel(x, coeffs, W, b, alpha):
    out, _ = _run(x, coeffs, W, b, alpha, trace=False)
    return out


# revision 17
# speedup vs baseline: 2.2355x; 1.0098x over previous
"""Trainium2 Bass kernel for MinimalKAN forward (nn_MinimalKAN_Normalized).

Math:
  a = sigmoid(alpha)
  out = (1-a) * (x @ W.T + b) + (a/sqrt(I)) * (x @ C0 + x^2 @ C1 + x^3 @ C2)

Folding the alpha blend into the weights on the host gives exactly
  out = x @ A + x^2 @ B + x^3 @ C + b_eff
with A = (1-a) W.T + s C0, B = s C1, C = s C2, b_eff = (1-a) b, s = a/sqrt(I).

Device strategy (data-parallel over batch, 8 cores), per core 4096 rows.
The contraction index i sits on SBUF partitions; the host feeds x^T in fp16.
Mixed precision split by term magnitude:
  - linear term x @ A: fp16 matmuls (A host-scaled by S16=64 to clear the
    fp16 subnormal range), 4 accumulating matmuls per 128-row tile.
  - kan terms x^2 @ B + x^3 @ C: fp8(e4m3) matmuls in DoubleRow perf mode:
    lhsT [128, 2, 128] loads two k-planes (each PE cell holds 2 weights),
    K=256 per instruction at 1 col/cycle -> 2x the fp16 MAC rate.  B,C are
    tiny (~2e-4) so fp8 error lands well under tolerance; host-scaled by
    4096 to clear fp8 subnormals.  TRN e4m3 saturates at +-240: max|x|=5.4
    -> max|x^3| ~ 160, safe.
  - x^2 on ACT (Square, fp8 out), x^3 on GpSimd (x^2*x), PSUM merges +
    bias on DVE, output stored bf16 (halves output DMA traffic).
Per 128-row tile PE cost: 4*512 (fp16) + 4*512 (fp8 DR) = 4096 cycles vs
12*512 = 6144 all-fp16: ~57us PE floor at 2.3 GHz.
All HBM tensors are host-relayouted to [128 partitions, ...contiguous] so
every DMA kick is 128 descriptors of 2-4KB (descriptor-issue rate on the
two HWDGE rings limits the pipeline head/tail otherwise).  Weights ride
the ACT ring, x^T the SP ring, outputs the ACT ring.
"""

import os
import numpy as np

import concourse.bass as bass
from concourse import bacc
import concourse.mybir as mybir
import concourse.tile as tile
from concourse.bass_utils import run_bass_kernel_spmd

N_CORES = 8
B, I, O = 32768, 512, 512
BS = B // N_CORES          # rows per core
P = 128
KS = I // P                # 4 contraction k-tiles per basis
G = int(os.environ.get("KAN_GROUP", "4"))     # 128-row tiles per group
GB = G * P                 # batch rows per group
N_GROUPS = BS // GB

S16 = 64.0                 # fp16 linear-weight host scale
S8A = 4096.0               # fp8 kan-weight host scale (x^2 and x^3 blocks)


def _build(repeat: int = 1) -> bass.Bass:
    f16 = mybir.dt.float16
    f8 = mybir.dt.float8e4
    f32 = mybir.dt.float32
    bf16 = mybir.dt.bfloat16
    sq = mybir.ActivationFunctionType.Square
    DR = mybir.MatmulPerfMode.DoubleRow
    mult = mybir.AluOpType.mult
    add = mybir.AluOpType.add

    nc = bacc.Bacc("TRN2", target_bir_lowering=False, debug=False,
                   num_devices=N_CORES)

    x_d = nc.dram_tensor("xt", [P, N_GROUPS, KS, GB], f16,
                         kind="ExternalInput")
    wl_d = nc.dram_tensor("wlin", [P, KS, O], f16, kind="ExternalInput")
    wk_d = nc.dram_tensor("wkan", [P, 2 * KS, O], f8, kind="ExternalInput")
    b_d = nc.dram_tensor("bias", [P, O], f16, kind="ExternalInput")
    o_d = nc.dram_tensor("out", [P, N_GROUPS, G, O], bf16,
                         kind="ExternalOutput")

    with tile.TileContext(nc) as tc:
        with (
            tc.tile_pool(name="const", bufs=1) as const,
            tc.tile_pool(name="xin", bufs=4) as xin,
            tc.tile_pool(name="basis", bufs=3) as basis,
            tc.tile_pool(name="outp", bufs=3) as outp,
            tc.tile_pool(name="tmp", bufs=4) as tmpp,
            tc.tile_pool(name="ps_l", bufs=3, space="PSUM") as ps_l,
            tc.tile_pool(name="ps_k", bufs=3, space="PSUM") as ps_k,
            tc.tile_pool(name="ps_w", bufs=1, space="PSUM") as ps_w,
        ):
            wl_sb = const.tile([P, KS, O], f16)
            nc.scalar.dma_start(wl_sb[:], wl_d[:])
            wk_sb = const.tile([P, 2 * KS, O], f8)
            nc.scalar.dma_start(wk_sb[:], wk_d[:])
            bsb = const.tile([P, O], f16)
            nc.scalar.dma_start(bsb[:], b_d[:])

            # Warm the PE p-state during the initial DMA fill: ~4us of
            # continuous dummy matmuls brings the clock to full speed
            # before the first real matmul arrives.
            warm = const.tile([P, P], f16)
            nc.vector.memset(warm[:], 0.0)
            po_w = ps_w.tile([P, P], f32, tag="po_w")
            for _ in range(24):
                nc.tensor.matmul(po_w[:], warm[:], warm[:],
                                 start=True, stop=True,
                                 skip_group_check=True)

            for g in [i for _ in range(repeat) for i in range(N_GROUPS)]:
                xT = xin.tile([P, KS, GB], f16, tag="xT")
                nc.sync.dma_start(xT[:], x_d[:, g])
                b8 = basis.tile([P, 2 * KS, GB], f8, tag="b8")
                o_sb = outp.tile([P, G, O], bf16, tag="o_sb")
                for j in range(G):
                    js = slice(j * P, (j + 1) * P)
                    nc.scalar.activation(b8[:, 0:KS, js], xT[:, :, js], sq)
                    nc.gpsimd.tensor_mul(
                        b8[:, KS:2 * KS, js], b8[:, 0:KS, js], xT[:, :, js])
                    po_l = ps_l.tile([P, O], f32, tag="po_l")
                    for k in range(KS):
                        nc.tensor.matmul(
                            po_l[:], xT[:, k, js], wl_sb[:, k, :],
                            start=(k == 0), stop=(k == KS - 1),
                            skip_group_check=True)
                    po_k = ps_k.tile([P, O], f32, tag="po_k")
                    for t in range(KS):
                        nc.tensor.matmul(
                            po_k[:],
                            b8[:, 2 * t:2 * t + 2, js],
                            wk_sb[:, 2 * t:2 * t + 2, :],
                            start=(t == 0), stop=(t == KS - 1),
                            perf_mode=DR, skip_group_check=True)
                    tmp = tmpp.tile([P, O], f32, tag="tmp")
                    nc.vector.scalar_tensor_tensor(
                        tmp[:], po_l[:], 1.0 / S16, bsb[:], mult, add)
                    nc.vector.scalar_tensor_tensor(
                        o_sb[:, j, :], po_k[:], 1.0 / S8A, tmp[:], mult, add)
                    if g == N_GROUPS - 1:
                        # last group: drain per-tile so the pipeline tail
                        # only waits on the final 128KB
                        nc.scalar.dma_start(o_d[:, g, j, :], o_sb[:, j, :])
                if g != N_GROUPS - 1:
                    nc.scalar.dma_start(o_d[:, g], o_sb[:])

    nc.compile()
    return nc


_NC_CACHE: dict[int, bass.Bass] = {}


def _get_nc(repeat: int = 1) -> bass.Bass:
    nc = _NC_CACHE.get(repeat)
    if nc is None:
        nc = _build(repeat)
        _NC_CACHE[repeat] = nc
    return nc


def _fold_weights(coeffs, W, b, alpha):
    a = 1.0 / (1.0 + np.exp(-np.float64(alpha)))
    s = a / np.sqrt(np.float64(I))
    A = (1.0 - a) * W.astype(np.float64).T + s * coeffs[:, :, 0].astype(np.float64)
    Bm = s * coeffs[:, :, 1].astype(np.float64)
    Cm = s * coeffs[:, :, 2].astype(np.float64)
    # [I, O] -> [P, KS, O] with row ks*P+p on partition p, slot ks
    wlin = (A * S16).astype(np.float16)
    wlin = np.ascontiguousarray(
        wlin.reshape(KS, P, O).transpose(1, 0, 2))
    f8np = mybir.dt.np(mybir.dt.float8e4)
    wkan = np.concatenate([Bm * S8A, Cm * S8A], axis=0)
    wkan = np.clip(wkan, -240.0, 240.0).astype(f8np)
    wkan = np.ascontiguousarray(
        wkan.reshape(2 * KS, P, O).transpose(1, 0, 2))
    b_eff = ((1.0 - a) * b.astype(np.float64)).astype(np.float16)
    bias_rep = np.ascontiguousarray(
        np.broadcast_to(b_eff[None, :], (P, O)))
    return wlin, wkan, bias_rep


def _make_in_maps(x, coeffs, W, b, alpha):
    wlin, wkan, bias_rep = _fold_weights(coeffs, W, b, alpha)
    x = np.asarray(x, dtype=np.float32)
    in_maps = []
    for c in range(N_CORES):
        shard = x[c * BS:(c + 1) * BS].astype(np.float16)
        # [BS, I] -> [P, N_GROUPS, KS, GB]: xt[p, g, ks, b'] =
        # x[g*GB+b', ks*P+p]
        xt = np.ascontiguousarray(
            shard.reshape(N_GROUPS, GB, KS, P).transpose(3, 0, 2, 1))
        in_maps.append({
            "wlin": wlin, "wkan": wkan, "bias": bias_rep, "xt": xt,
        })
    return in_maps


def _unpack_out(raw):
    # [P, N_GROUPS, G, O] bf16 -> [BS, O] f32: row g*GB + j*P + p
    return np.ascontiguousarray(
        np.asarray(raw).astype(np.float32).transpose(1, 2, 0, 3)
    ).reshape(BS, O)


def _run(x, coeffs, W, b, alpha, trace=False):
    nc = _get_nc()
    in_maps = _make_in_maps(x, coeffs, W, b, alpha)
    res = run_bass_kernel_spmd(nc, in_maps, core_ids=list(range(N_CORES)),
                               trace=trace)
    out = np.concatenate([_unpack_out(r["out"]) for r in res.results], axis=0)
    return out, res


def kernel(x, coeffs, W, b, alpha):
    out, _ = _run(x, coeffs, W, b, alpha, trace=False)
    return out
